# revision 6
# baseline (speedup 1.0000x reference)
"""AFiReHead (DINO-style head) distributed Bass kernel for 8 TRN2 NeuronCores.

Reference computation:
    h  = gelu(x @ W1.T + b1)          x: [B=64, P=196, IN=768], W1: [1024, 768]
    h  = gelu(h @ W2.T + b2)          W2: [1024, 1024]
    z  = h @ W3.T + b3                W3: [256, 1024]
    z  = z / max(||z||_2, eps)        over last dim (256)
    W  = g/||v|| * v                  v: [196, 4096, 256] per-patch weight-norm heads
    out[b,p,o] = z[b,p,:] . W[p,o,:]  -> [64, 196, 4096]

Sharding: expert-parallel over the patch axis. P padded 196 -> 208, 26
patches per core; each core runs the full pipeline for its patches with the
small MLP weights replicated. No collectives needed - each core produces an
independent output slice which the host concatenates.

Device layout is feature-major ([feature, token], token = patch*64 + batch)
so activations chain through the three matmuls and the per-patch head with
no on-device transposes; all transposes are host-side numpy views.

Numerics: bf16 matmul operands with f32 PSUM accumulation (verified
rel2err ~5e-3 vs f32 reference). The weight-norm scale g/||v|| is folded
into v host-side (standard inference-time weight-norm folding); the
data-dependent L2 norm of z is computed on device and applied as a
per-partition scale in the head epilogue.
"""

import os
from contextlib import ExitStack

import numpy as np
import ml_dtypes

import concourse.bass as bass
import concourse.tile as tile
from concourse import bacc, mybir
from concourse.bass import ts
from concourse.bass_utils import run_bass_kernel_spmd

BF16 = mybir.dt.bfloat16
F32 = mybir.dt.float32
AF = mybir.ActivationFunctionType

B = 64          # batch
P_FULL = 196    # patches
IN = 768
H = 1024
BT = 256        # bottleneck
OUT = 4096
N_CORES = 8
NP = 26         # patches per core (196 padded to 208)
T = NP * B      # tokens per core, ordered t = p*64 + b


def build_kernel(np_patches=NP, nch=4, num_devices=N_CORES, debug=False, act=AF.Gelu):
    """Build the per-core Bass graph. All 8 cores run this same graph SPMD."""
    t_tok = np_patches * B           # tokens this core
    n_pairs = t_tok // 128           # patch pairs == 128-token tiles
    assert t_tok % 128 == 0 and t_tok % nch == 0
    ch = t_tok // nch                # MLP free-dim chunk (<=512 for f32 PSUM)
    assert ch <= 512
    KI, KH, KZ = IN // 128, H // 128, BT // 128   # 6, 8, 2 k-tiles

    nc = bacc.Bacc(
        "TRN2", target_bir_lowering=False, debug=debug, num_devices=num_devices
    )
    xt_d = nc.dram_tensor("xt", [IN, t_tok], BF16, kind="ExternalInput").ap()
    w1_d = nc.dram_tensor("w1t", [IN, H], BF16, kind="ExternalInput").ap()
    w2_d = nc.dram_tensor("w2t", [H, H], BF16, kind="ExternalInput").ap()
    w3_d = nc.dram_tensor("w3t", [H, BT], BF16, kind="ExternalInput").ap()
    b1_d = nc.dram_tensor("b1r", [128, KH], F32, kind="ExternalInput").ap()
    b2_d = nc.dram_tensor("b2r", [128, KH], F32, kind="ExternalInput").ap()
    b3_d = nc.dram_tensor("b3r", [128, KZ], F32, kind="ExternalInput").ap()
    vt_d = nc.dram_tensor("vt", [np_patches, BT, OUT], BF16, kind="ExternalInput").ap()
    out_d = nc.dram_tensor("out", [np_patches, B, OUT], BF16, kind="ExternalOutput").ap()

    with tile.TileContext(nc) as tc, ExitStack() as ctx:
        const = ctx.enter_context(tc.tile_pool(name="const", bufs=1))
        acts = ctx.enter_context(tc.tile_pool(name="acts", bufs=1))
        vpool = ctx.enter_context(tc.tile_pool(name="vpool", bufs=4))
        mmps = ctx.enter_context(tc.tile_pool(name="mmps", bufs=6, space="PSUM"))
        nps_p = ctx.enter_context(tc.tile_pool(name="npsum", bufs=2, space="PSUM"))
        outp = ctx.enter_context(tc.tile_pool(name="outp", bufs=8))
        small = ctx.enter_context(tc.tile_pool(name="small", bufs=2))

        # --- resident loads -------------------------------------------------
        w1s = const.tile([128, KI, H], BF16)
        nc.sync.dma_start(w1s, w1_d.rearrange("(a p) n -> p a n", p=128))
        xts = acts.tile([128, KI, t_tok], BF16)
        nc.sync.dma_start(xts, xt_d.rearrange("(a p) n -> p a n", p=128))
        w2s = const.tile([128, KH, H], BF16)
        nc.sync.dma_start(w2s, w2_d.rearrange("(a p) n -> p a n", p=128))
        w3s = const.tile([128, KH, BT], BF16)
        nc.sync.dma_start(w3s, w3_d.rearrange("(a p) n -> p a n", p=128))
        b1s = const.tile([128, KH], F32)
        nc.sync.dma_start(b1s, b1_d)
        b2s = const.tile([128, KH], F32)
        nc.sync.dma_start(b2s, b2_d)
        b3s = const.tile([128, KZ], F32)
        nc.sync.dma_start(b3s, b3_d)
        ones = const.tile([128, 1], BF16)
        nc.any.memset(ones, 1.0)
        eps2 = const.tile([128, 1], F32)
        nc.any.memset(eps2, 1e-24)

        h1 = acts.tile([128, KH, t_tok], BF16)
        h2 = acts.tile([128, KH, t_tok], BF16)
        zt = acts.tile([128, KZ, t_tok], BF16)
        zsq = acts.tile([128, KZ, t_tok], BF16)
        rn = acts.tile([128, n_pairs], F32)

        # prefetch v for the first pairs while the MLP runs
        vtiles = []
        for p in range(min(4, np_patches)):
            vtl = vpool.tile([128, KZ, OUT], BF16, tag="v")
            nc.sync.dma_start(vtl, vt_d[p].rearrange("(a q) n -> q a n", q=128))
            vtiles.append(vtl)

        # --- MLP ------------------------------------------------------------
        def layer(src, wts, kt, mt, bias, dst, func):
            for m in range(mt):
                for n in range(nch):
                    ps = mmps.tile([128, ch], F32, tag="mm", padded_shape=[128, 512])
                    for k in range(kt):
                        nc.tensor.matmul(
                            ps,
                            wts[:, k, ts(m, 128)],
                            src[:, k, ts(n, ch)],
                            start=(k == 0),
                            stop=(k == kt - 1),
                        )
                    nc.scalar.activation(
                        dst[:, m, ts(n, ch)], ps, func, bias=bias[:, m : m + 1]
                    )

        layer(xts, w1s, KI, KH, b1s, h1, act)
        layer(h1, w2s, KH, KH, b2s, h2, act)

        # z = W3T.T @ h2 + b3 ; also z^2 for the norm
        for m in range(KZ):
            for n in range(nch):
                ps = mmps.tile([128, ch], F32, tag="mm", padded_shape=[128, 512])
                for k in range(KH):
                    nc.tensor.matmul(
                        ps,
                        w3s[:, k, ts(m, 128)],
                        h2[:, k, ts(n, ch)],
                        start=(k == 0),
                        stop=(k == KH - 1),
                    )
                nc.scalar.activation(
                    zt[:, m, ts(n, ch)], ps, AF.Identity, bias=b3s[:, m : m + 1]
                )
                nc.scalar.activation(
                    zsq[:, m, ts(n, ch)], ps, AF.Square, bias=b3s[:, m : m + 1]
                )

        # nsq[t] = sum_d z^2 (token-major via ones-matmul), rn = 1/sqrt(nsq)
        for j in range(n_pairs):
            nq = nps_p.tile([128, 1], F32, tag="nsq")
            for k in range(KZ):
                nc.tensor.matmul(
                    nq, zsq[:, k, ts(j, 128)], ones, start=(k == 0), stop=(k == KZ - 1)
                )
            sj = small.tile([128, 1], F32, tag="s")
            nc.scalar.activation(sj, nq, AF.Sqrt, bias=eps2[:, 0:1])
            nc.vector.reciprocal(rn[:, j : j + 1], sj)

        # --- per-patch heads (col-packed pairs) -----------------------------
        for j in range(n_pairs):
            pa, pb = 2 * j, 2 * j + 1
            if pa < len(vtiles):
                va, vb = vtiles[pa], vtiles[pb]
            else:
                va = vpool.tile([128, KZ, OUT], BF16, tag="v")
                nc.sync.dma_start(va, vt_d[pa].rearrange("(a q) n -> q a n", q=128))
                vb = vpool.tile([128, KZ, OUT], BF16, tag="v")
                nc.sync.dma_start(vb, vt_d[pb].rearrange("(a q) n -> q a n", q=128))
            for c in range(OUT // 512):
                hp = mmps.tile([128, 512], F32, tag="mm")
                # AABB order: groups in this bank never interleave, while the
                # A->B col-group transition still overlaps in the PE array.
                for half, pp, vv in ((0, pa, va), (64, pb, vb)):
                    for k in range(KZ):
                        nc.tensor.matmul(
                            hp[half : half + 64, :],
                            zt[:, k, ts(pp, 64)],
                            vv[:, k, ts(c, 512)],
                            start=(k == 0),
                            stop=(k == KZ - 1),
                        )
                ob = outp.tile([128, 512], BF16, tag="ob")
                nc.vector.tensor_scalar_mul(ob, hp, rn[:, j : j + 1])
                nc.sync.dma_start(
                    out_d[pa : pb + 1, :, ts(c, 512)].rearrange("a b n -> (a b) n"),
                    ob,
                )

    nc.compile()
    return nc


def shard_inputs(x, W1, b1, W2, b2, W3, b3, v, g, np_patches=NP):
    """Host-side: pad P, fold g/||v|| into v, transpose to device layouts."""
    bf = ml_dtypes.bfloat16
    p_pad = np_patches * N_CORES

    vn = np.sqrt(np.sum(v * v, axis=-1, keepdims=True))
    vs = (v * (g[..., None] / vn)).astype(bf)            # [196, 4096, 256]
    vs_t = np.ascontiguousarray(vs.transpose(0, 2, 1))   # [196, 256, 4096]
    vt_all = np.zeros((p_pad, BT, OUT), dtype=bf)
    vt_all[:P_FULL] = vs_t

    xp = np.zeros((p_pad, B, IN), dtype=np.float32)      # token order (p, b)
    xp[:P_FULL] = x.transpose(1, 0, 2)

    shared = {
        "w1t": np.ascontiguousarray(W1.T).astype(bf),
        "w2t": np.ascontiguousarray(W2.T).astype(bf),
        "w3t": np.ascontiguousarray(W3.T).astype(bf),
        "b1r": np.ascontiguousarray(b1.reshape(-1, 128).T).astype(np.float32),
        "b2r": np.ascontiguousarray(b2.reshape(-1, 128).T).astype(np.float32),
        "b3r": np.ascontiguousarray(b3.reshape(-1, 128).T).astype(np.float32),
    }
    t_tok = np_patches * B
    in_maps = []
    for c in range(N_CORES):
        sl = slice(c * np_patches, (c + 1) * np_patches)
        xt = np.ascontiguousarray(xp[sl].reshape(t_tok, IN).T).astype(bf)
        in_maps.append({"xt": xt, "vt": vt_all[sl], **shared})
    return in_maps


def unshard_output(results, np_patches=NP):
    outs = [np.asarray(r["out"], dtype=np.float32) for r in results]
    full = np.concatenate(outs, axis=0)                  # [208, 64, 4096]
    return np.ascontiguousarray(full[:P_FULL].transpose(1, 0, 2))


_NC_CACHE = {}


def _get_nc():
    if "nc" not in _NC_CACHE:
        _NC_CACHE["nc"] = build_kernel()
    return _NC_CACHE["nc"]


def kernel(x, W1, b1, W2, b2, W3, b3, v, g):
    x = np.asarray(x, dtype=np.float32)
    in_maps = shard_inputs(
        x,
        np.asarray(W1, dtype=np.float32),
        np.asarray(b1, dtype=np.float32),
        np.asarray(W2, dtype=np.float32),
        np.asarray(b2, dtype=np.float32),
        np.asarray(W3, dtype=np.float32),
        np.asarray(b3, dtype=np.float32),
        np.asarray(v, dtype=np.float32),
        np.asarray(g, dtype=np.float32),
    )
    nc = _get_nc()
    res = run_bass_kernel_spmd(nc, in_maps, core_ids=list(range(N_CORES)))
    return unshard_output(res.results)


# revision 10
# speedup vs baseline: 1.3685x; 1.3685x over previous
"""AFiReHead (DINO-style head) distributed Bass kernel for 8 TRN2 NeuronCores.

Reference computation:
    h  = gelu(x @ W1.T + b1)          x: [B=64, P=196, IN=768], W1: [1024, 768]
    h  = gelu(h @ W2.T + b2)          W2: [1024, 1024]
    z  = h @ W3.T + b3                W3: [256, 1024]
    z  = z / max(||z||_2, eps)        over last dim (256)
    W  = g/||v|| * v                  v: [196, 4096, 256] per-patch weight-norm heads
    out[b,p,o] = z[b,p,:] . W[p,o,:]  -> [64, 196, 4096]

Sharding: expert-parallel over the patch axis. P padded 196 -> 208, 26
patches per core; each core runs the full pipeline for its patches with the
small MLP weights replicated. No collectives needed - each core produces an
independent output slice which the host concatenates.

Device layout is feature-major ([feature, token], token = patch*64 + batch)
so activations chain through the three matmuls and the per-patch head with
no on-device transposes; all transposes are host-side numpy views.

Numerics: bf16 matmul operands with f32 PSUM accumulation (verified
rel2err ~5e-3 vs f32 reference). The weight-norm scale g/||v|| is folded
into v host-side (standard inference-time weight-norm folding); the
data-dependent L2 norm of z is computed on device and applied as a
per-partition scale in the head epilogue.

Schedule: the wall-clock floor is the v stream (~54 MB/core). A small MLP
prologue (2 patch pairs) produces the first z tiles, then the remaining MLP
work is emitted as filler units interleaved between per-pair head blocks so
the v DMA stream never stalls on buffer space and the PE never idles long
enough to re-throttle (HAM).
"""

from contextlib import ExitStack

import numpy as np
import ml_dtypes

import concourse.bass as bass
import concourse.tile as tile
from concourse import bacc, mybir
from concourse.bass import ds, ts
from concourse.bass_utils import run_bass_kernel_spmd

BF16 = mybir.dt.bfloat16
F32 = mybir.dt.float32
AF = mybir.ActivationFunctionType

B = 64          # batch
P_FULL = 196    # patches
IN = 768
H = 1024
BT = 256        # bottleneck
OUT = 4096
N_CORES = 8
NP = 26         # patches per core (196 padded to 208)
T = NP * B      # tokens per core, ordered t = p*64 + b


def _nch(tokens):
    n = max(1, (tokens + 511) // 512)
    while tokens % n:
        n += 1
    return n


def build_kernel(
    np_patches=NP,
    num_devices=N_CORES,
    debug=False,
    act=AF.Gelu,
    pairs_a=2,
    v_bufs=4,
    warmup_mms=48,
):
    """Build the per-core Bass graph. All 8 cores run this same graph SPMD."""
    t_tok = np_patches * B
    n_pairs = t_tok // 128
    sections = []
    rem = n_pairs
    while rem:
        s = min(pairs_a if len(sections) < 2 else pairs_a + 1, rem)
        sections.append(s)
        rem -= s
    bounds = []
    q = 0
    for s in sections:
        bounds.append((q, q + s))
        q += s
    KI, KH, KZ = IN // 128, H // 128, BT // 128   # 6, 8, 2 k-tiles

    nc = bacc.Bacc(
        "TRN2", target_bir_lowering=False, debug=debug, num_devices=num_devices
    )
    xt_d = nc.dram_tensor("xt", [IN, t_tok], BF16, kind="ExternalInput").ap()
    w1_d = nc.dram_tensor("w1t", [IN, H], BF16, kind="ExternalInput").ap()
    w2_d = nc.dram_tensor("w2t", [H, H], BF16, kind="ExternalInput").ap()
    w3_d = nc.dram_tensor("w3t", [H, BT], BF16, kind="ExternalInput").ap()
    b1_d = nc.dram_tensor("b1r", [128, KH], F32, kind="ExternalInput").ap()
    b2_d = nc.dram_tensor("b2r", [128, KH], F32, kind="ExternalInput").ap()
    b3_d = nc.dram_tensor("b3r", [128, KZ], F32, kind="ExternalInput").ap()
    vt_d = nc.dram_tensor("vt", [np_patches, BT, OUT], BF16, kind="ExternalInput").ap()
    out_d = nc.dram_tensor("out", [np_patches, B, OUT], BF16, kind="ExternalOutput").ap()

    with tile.TileContext(nc) as tc, ExitStack() as ctx:
        const = ctx.enter_context(tc.tile_pool(name="const", bufs=1))
        acts = ctx.enter_context(tc.tile_pool(name="acts", bufs=1))
        vpool = ctx.enter_context(tc.tile_pool(name="vpool", bufs=v_bufs))
        mmps = ctx.enter_context(tc.tile_pool(name="mmps", bufs=6, space="PSUM"))
        nps_p = ctx.enter_context(tc.tile_pool(name="npsum", bufs=2, space="PSUM"))
        outp = ctx.enter_context(tc.tile_pool(name="outp", bufs=4))
        small = ctx.enter_context(tc.tile_pool(name="small", bufs=2))

        # --- PE warmup: dummy matmuls on constants while inputs stream in ---
        wz = const.tile([128, 512], BF16, name="wz")
        nc.any.memset(wz, 0.0)
        for i in range(warmup_mms):
            wp = mmps.tile([128, 512], F32, tag="mm", name=f"warm{i}")
            nc.tensor.matmul(wp, wz[:, 0:128], wz, start=True, stop=True)

        # --- resident loads -------------------------------------------------
        w1s = const.tile([128, KI, H], BF16)
        nc.sync.dma_start(w1s, w1_d.rearrange("(a p) n -> p a n", p=128))
        xts = acts.tile([128, KI, t_tok], BF16)
        nc.sync.dma_start(xts, xt_d.rearrange("(a p) n -> p a n", p=128))
        b1s = const.tile([128, KH], F32)
        nc.sync.dma_start(b1s, b1_d)
        w2s = const.tile([128, KH, H], BF16)
        nc.sync.dma_start(w2s, w2_d.rearrange("(a p) n -> p a n", p=128))
        w3s = const.tile([128, KH, BT], BF16)
        nc.sync.dma_start(w3s, w3_d.rearrange("(a p) n -> p a n", p=128))
        b2s = const.tile([128, KH], F32)
        nc.sync.dma_start(b2s, b2_d)
        b3s = const.tile([128, KZ], F32)
        nc.sync.dma_start(b3s, b3_d)
        ones = const.tile([128, 1], BF16)
        nc.any.memset(ones, 1.0)
        eps2 = const.tile([128, 1], F32)
        nc.any.memset(eps2, 1e-24)

        # per-section activation tiles (cross-section reads and writes must
        # never share a tile: program order defines semantics under Tile)
        hts = []
        for hi, (q0, q1) in enumerate(bounds):
            tkn = (q1 - q0) * 128
            hts.append(
                {
                    "h1": acts.tile([128, KH, tkn], BF16, name=f"h1_{hi}"),
                    "h2": acts.tile([128, KH, tkn], BF16, name=f"h2_{hi}"),
                    "zt": acts.tile([128, KZ, tkn], BF16, name=f"zt_{hi}"),
                    "zsq": acts.tile([128, KZ, tkn], BF16, name=f"zsq_{hi}"),
                    "rn": acts.tile([128, q1 - q0], F32, name=f"rn_{hi}"),
                }
            )

        # --- v stream (per-patch tiles, issued ~2 pairs ahead) --------------
        vt_tiles = {}

        def issue_v(p):
            if p >= np_patches or p in vt_tiles:
                return
            vtl = vpool.tile([128, KZ, OUT], BF16, tag="v", name=f"v{p}")
            nc.sync.dma_start(vtl, vt_d[p].rearrange("(a q) n -> q a n", q=128))
            vt_tiles[p] = vtl

        for p in range(min(v_bufs, np_patches)):
            issue_v(p)

        # --- MLP unit generators (one unit = one m-tile through all k) ------
        def mlp_units(hi):
            q0, q1 = bounds[hi]
            tkn = (q1 - q0) * 128
            tok0 = q0 * 128
            nch = _nch(tkn)
            ch = tkn // nch
            ht = hts[hi]

            def mm_unit(src, src_glob, wts, kt, m, evict):
                def f():
                    pss = [
                        mmps.tile(
                            [128, ch],
                            F32,
                            tag="mm",
                            padded_shape=[128, 512],
                            name=f"ps{m}_{n}",
                        )
                        for n in range(nch)
                    ]
                    for k in range(kt):
                        for n in range(nch):
                            rhs = (
                                src[:, k, ds(tok0 + n * ch, ch)]
                                if src_glob
                                else src[:, k, ds(n * ch, ch)]
                            )
                            nc.tensor.matmul(
                                pss[n],
                                wts[:, k, ts(m, 128)],
                                rhs,
                                start=(k == 0),
                                stop=(k == kt - 1),
                            )
                    for n in range(nch):
                        evict(pss[n], n * ch, ch, m)

                return f

            def ev_h1(ps, o, c, m):
                nc.scalar.activation(
                    ht["h1"][:, m, ds(o, c)], ps, act, bias=b1s[:, m : m + 1]
                )

            def ev_h2(ps, o, c, m):
                nc.scalar.activation(
                    ht["h2"][:, m, ds(o, c)], ps, act, bias=b2s[:, m : m + 1]
                )

            def ev_z(ps, o, c, m):
                nc.scalar.activation(
                    ht["zt"][:, m, ds(o, c)], ps, AF.Identity, bias=b3s[:, m : m + 1]
                )
                nc.scalar.activation(
                    ht["zsq"][:, m, ds(o, c)], ps, AF.Square, bias=b3s[:, m : m + 1]
                )

            units = [mm_unit(xts, True, w1s, KI, m, ev_h1) for m in range(KH)]
            units += [mm_unit(ht["h1"], False, w2s, KH, m, ev_h2) for m in range(KH)]
            units += [mm_unit(ht["h2"], False, w3s, KH, m, ev_z) for m in range(KZ)]

            def nsq_unit(lj):
                def f():
                    nq = nps_p.tile([128, 1], F32, tag="nsq")
                    for k in range(KZ):
                        nc.tensor.matmul(
                            nq,
                            ht["zsq"][:, k, ts(lj, 128)],
                            ones,
                            start=(k == 0),
                            stop=(k == KZ - 1),
                        )
                    sj = small.tile([128, 1], F32, tag="s")
                    nc.scalar.activation(sj, nq, AF.Sqrt, bias=eps2[:, 0:1])
                    nc.vector.reciprocal(ht["rn"][:, lj : lj + 1], sj)

                return f

            units += [nsq_unit(lj) for lj in range(q1 - q0)]
            return units

        # --- head block for one patch pair ----------------------------------
        def head_pair(j):
            hi = next(i for i, (q0, q1) in enumerate(bounds) if q0 <= j < q1)
            lj = j - bounds[hi][0]
            ht = hts[hi]
            va = vt_tiles.pop(2 * j)
            vb = vt_tiles.pop(2 * j + 1)
            ob = None
            for c in range(OUT // 512):
                hp = mmps.tile([128, 512], F32, tag="mm")
                for half, vv in ((0, va), (64, vb)):
                    for k in range(KZ):
                        nc.tensor.matmul(
                            hp[half : half + 64, :],
                            ht["zt"][:, k, ds(lj * 128 + half, 64)],
                            vv[:, k, ts(c, 512)],
                            start=(k == 0),
                            stop=(k == KZ - 1),
                        )
                if c % 2 == 0:
                    ob = outp.tile([128, 1024], BF16, tag="ob")
                nc.vector.tensor_scalar_mul(
                    ob[:, ts(c % 2, 512)], hp, ht["rn"][:, lj : lj + 1]
                )
                if c % 2 == 1:
                    nc.sync.dma_start(
                        out_d[2 * j : 2 * j + 2, :, ds((c - 1) * 512, 1024)].rearrange(
                            "a b n -> (a b) n"
                        ),
                        ob,
                    )

        # --- emit: software pipeline over sections --------------------------
        # MLP(section 0); then for each section s: its head pairs with
        # MLP(section s+1) units distributed between them as PE filler.
        for u in mlp_units(0):
            u()
        for si, (q0, q1) in enumerate(bounds):
            nxt = mlp_units(si + 1) if si + 1 < len(bounds) else []
            emitted = 0
            for idx, j in enumerate(range(q0, q1)):
                head_pair(j)
                issue_v(2 * (j + 2))
                issue_v(2 * (j + 2) + 1)
                target = round((idx + 1) * len(nxt) / (q1 - q0))
                while emitted < target:
                    nxt[emitted]()
                    emitted += 1

    nc.compile()
    return nc


def shard_inputs(x, W1, b1, W2, b2, W3, b3, v, g, np_patches=NP):
    """Host-side: pad P, fold g/||v|| into v, transpose to device layouts."""
    bf = ml_dtypes.bfloat16
    p_pad = np_patches * N_CORES

    vn = np.sqrt(np.sum(v * v, axis=-1, keepdims=True))
    vs = (v * (g[..., None] / vn)).astype(bf)            # [196, 4096, 256]
    vs_t = np.ascontiguousarray(vs.transpose(0, 2, 1))   # [196, 256, 4096]
    vt_all = np.zeros((p_pad, BT, OUT), dtype=bf)
    vt_all[:P_FULL] = vs_t

    xp = np.zeros((p_pad, B, IN), dtype=np.float32)      # token order (p, b)
    xp[:P_FULL] = x.transpose(1, 0, 2)

    shared = {
        "w1t": np.ascontiguousarray(W1.T).astype(bf),
        "w2t": np.ascontiguousarray(W2.T).astype(bf),
        "w3t": np.ascontiguousarray(W3.T).astype(bf),
        "b1r": np.ascontiguousarray(b1.reshape(-1, 128).T).astype(np.float32),
        "b2r": np.ascontiguousarray(b2.reshape(-1, 128).T).astype(np.float32),
        "b3r": np.ascontiguousarray(b3.reshape(-1, 128).T).astype(np.float32),
    }
    t_tok = np_patches * B
    in_maps = []
    for c in range(N_CORES):
        sl = slice(c * np_patches, (c + 1) * np_patches)
        xt = np.ascontiguousarray(xp[sl].reshape(t_tok, IN).T).astype(bf)
        in_maps.append({"xt": xt, "vt": vt_all[sl], **shared})
    return in_maps


def unshard_output(results, np_patches=NP):
    outs = [np.asarray(r["out"], dtype=np.float32) for r in results]
    full = np.concatenate(outs, axis=0)                  # [208, 64, 4096]
    return np.ascontiguousarray(full[:P_FULL].transpose(1, 0, 2))


_NC_CACHE = {}


def _get_nc():
    if "nc" not in _NC_CACHE:
        _NC_CACHE["nc"] = build_kernel()
    return _NC_CACHE["nc"]


def kernel(x, W1, b1, W2, b2, W3, b3, v, g):
    x = np.asarray(x, dtype=np.float32)
    in_maps = shard_inputs(
        x,
        np.asarray(W1, dtype=np.float32),
        np.asarray(b1, dtype=np.float32),
        np.asarray(W2, dtype=np.float32),
        np.asarray(b2, dtype=np.float32),
        np.asarray(W3, dtype=np.float32),
        np.asarray(b3, dtype=np.float32),
        np.asarray(v, dtype=np.float32),
        np.asarray(g, dtype=np.float32),
    )
    nc = _get_nc()
    res = run_bass_kernel_spmd(nc, in_maps, core_ids=list(range(N_CORES)))
    return unshard_output(res.results)


# revision 13
# speedup vs baseline: 1.5387x; 1.1244x over previous
"""AFiReHead (DINO-style head) distributed Bass kernel for 8 TRN2 NeuronCores.

Reference computation:
    h  = gelu(x @ W1.T + b1)          x: [B=64, P=196, IN=768], W1: [1024, 768]
    h  = gelu(h @ W2.T + b2)          W2: [1024, 1024]
    z  = h @ W3.T + b3                W3: [256, 1024]
    z  = z / max(||z||_2, eps)        over last dim (256)
    W  = g/||v|| * v                  v: [196, 4096, 256] per-patch weight-norm heads
    out[b,p,o] = z[b,p,:] . W[p,o,:]  -> [64, 196, 4096]

Sharding: expert-parallel over the patch axis. P padded 196 -> 208, 26
patches per core; each core runs the full pipeline for its patches with the
small MLP weights replicated. No collectives needed - each core produces an
independent output slice which the host concatenates.

Device layout is feature-major ([feature, token], token = patch*64 + batch)
so activations chain through the three matmuls and the per-patch head with
no on-device transposes; all transposes are host-side numpy views.

Numerics: bf16 matmul operands with f32 PSUM accumulation (verified
rel2err ~5e-3 vs f32 reference). The weight-norm scale g/||v|| is folded
into v host-side (standard inference-time weight-norm folding); the
data-dependent L2 norm of z is computed on device and applied as a
per-partition scale in the head epilogue.

Schedule: the wall-clock floor is the v stream (~54 MB/core). A small MLP
prologue (2 patch pairs) produces the first z tiles, then the remaining MLP
work is emitted as filler units interleaved between per-pair head blocks so
the v DMA stream never stalls on buffer space and the PE never idles long
enough to re-throttle (HAM).
"""

from contextlib import ExitStack

import numpy as np
import ml_dtypes

import concourse.bass as bass
import concourse.tile as tile
from concourse import bacc, mybir
from concourse.bass import ds, ts
from concourse.bass_utils import run_bass_kernel_spmd

BF16 = mybir.dt.bfloat16
F32 = mybir.dt.float32
AF = mybir.ActivationFunctionType

B = 64          # batch
P_FULL = 196    # patches
IN = 768
H = 1024
BT = 256        # bottleneck
OUT = 4096
N_CORES = 8
NP = 26         # patches per core (196 padded to 208)
T = NP * B      # tokens per core, ordered t = p*64 + b


def _nch(tokens):
    n = max(1, (tokens + 511) // 512)
    while tokens % n:
        n += 1
    return n


def build_kernel(
    np_patches=NP,
    num_devices=N_CORES,
    debug=False,
    act=AF.Gelu,
    pairs_a=2,
    v_bufs=4,
    warmup_mms=48,
):
    """Build the per-core Bass graph. All 8 cores run this same graph SPMD."""
    t_tok = np_patches * B
    n_pairs = t_tok // 128
    sections = []
    rem = n_pairs
    while rem:
        s = min(pairs_a if len(sections) < 2 else pairs_a + 1, rem)
        sections.append(s)
        rem -= s
    bounds = []
    q = 0
    for s in sections:
        bounds.append((q, q + s))
        q += s
    KI, KH, KZ = IN // 128, H // 128, BT // 128   # 6, 8, 2 k-tiles

    nc = bacc.Bacc(
        "TRN2", target_bir_lowering=False, debug=debug, num_devices=num_devices
    )
    xt_d = nc.dram_tensor("xt", [IN, t_tok], BF16, kind="ExternalInput").ap()
    w1_d = nc.dram_tensor("w1t", [IN, H], BF16, kind="ExternalInput").ap()
    w2_d = nc.dram_tensor("w2t", [H, H], BF16, kind="ExternalInput").ap()
    w3_d = nc.dram_tensor("w3t", [H, BT], BF16, kind="ExternalInput").ap()
    b1_d = nc.dram_tensor("b1r", [128, KH], F32, kind="ExternalInput").ap()
    b2_d = nc.dram_tensor("b2r", [128, KH], F32, kind="ExternalInput").ap()
    b3_d = nc.dram_tensor("b3r", [128, KZ], F32, kind="ExternalInput").ap()
    vt_d = nc.dram_tensor("vt", [np_patches, BT, OUT], BF16, kind="ExternalInput").ap()
    out_d = nc.dram_tensor("out", [np_patches, B, OUT], BF16, kind="ExternalOutput").ap()

    with tile.TileContext(nc) as tc, ExitStack() as ctx:
        const = ctx.enter_context(tc.tile_pool(name="const", bufs=1))
        acts = ctx.enter_context(tc.tile_pool(name="acts", bufs=1))
        vpool = ctx.enter_context(tc.tile_pool(name="vpool", bufs=v_bufs))
        mmps = ctx.enter_context(tc.tile_pool(name="mmps", bufs=6, space="PSUM"))
        nps_p = ctx.enter_context(tc.tile_pool(name="npsum", bufs=2, space="PSUM"))
        outp = ctx.enter_context(tc.tile_pool(name="outp", bufs=4))
        small = ctx.enter_context(tc.tile_pool(name="small", bufs=2))

        # --- PE warmup: dummy matmuls on constants while inputs stream in ---
        wz = const.tile([128, 512], BF16, name="wz")
        nc.any.memset(wz, 0.0)
        for i in range(warmup_mms):
            wp = mmps.tile([128, 512], F32, tag="mm", name=f"warm{i}")
            nc.tensor.matmul(wp, wz[:, 0:128], wz, start=True, stop=True)

        # --- resident loads -------------------------------------------------
        w1s = const.tile([128, KI, H], BF16)
        nc.sync.dma_start(w1s, w1_d.rearrange("(a p) n -> p a n", p=128))
        xts = acts.tile([128, KI, t_tok], BF16)
        nc.sync.dma_start(xts, xt_d.rearrange("(a p) n -> p a n", p=128))
        b1s = const.tile([128, KH], F32)
        nc.sync.dma_start(b1s, b1_d)
        w2s = const.tile([128, KH, H], BF16)
        nc.sync.dma_start(w2s, w2_d.rearrange("(a p) n -> p a n", p=128))
        w3s = const.tile([128, KH, BT], BF16)
        nc.sync.dma_start(w3s, w3_d.rearrange("(a p) n -> p a n", p=128))
        b2s = const.tile([128, KH], F32)
        nc.sync.dma_start(b2s, b2_d)
        b3s = const.tile([128, KZ], F32)
        nc.sync.dma_start(b3s, b3_d)
        ones = const.tile([128, 1], BF16)
        nc.any.memset(ones, 1.0)
        eps2 = const.tile([128, 1], F32)
        nc.any.memset(eps2, 1e-24)

        # per-section activation tiles (cross-section reads and writes must
        # never share a tile: program order defines semantics under Tile)
        hts = []
        for hi, (q0, q1) in enumerate(bounds):
            tkn = (q1 - q0) * 128
            hts.append(
                {
                    "h1": acts.tile([128, KH, tkn], BF16, name=f"h1_{hi}"),
                    "h2": acts.tile([128, KH, tkn], BF16, name=f"h2_{hi}"),
                    "zt": acts.tile([128, KZ, tkn], BF16, name=f"zt_{hi}"),
                    "zsq": acts.tile([128, KZ, tkn], BF16, name=f"zsq_{hi}"),
                    "rn": acts.tile([128, q1 - q0], F32, name=f"rn_{hi}"),
                }
            )

        # --- v stream (per-patch tiles, issued ~2 pairs ahead) --------------
        vt_tiles = {}

        def issue_v(p):
            if p >= np_patches or p in vt_tiles:
                return
            vtl = vpool.tile([128, KZ, OUT], BF16, tag="v", name=f"v{p}")
            nc.sync.dma_start(vtl, vt_d[p].rearrange("(a q) n -> q a n", q=128))
            vt_tiles[p] = vtl

        for p in range(min(v_bufs, np_patches)):
            issue_v(p)

        # --- MLP unit generators (one unit = one m-tile through all k) ------
        def mlp_units(hi):
            q0, q1 = bounds[hi]
            tkn = (q1 - q0) * 128
            tok0 = q0 * 128
            nch = _nch(tkn)
            ch = tkn // nch
            ht = hts[hi]

            def mm_unit(src, src_glob, wts, kt, m, evict):
                def f():
                    pss = [
                        mmps.tile(
                            [128, ch],
                            F32,
                            tag="mm",
                            padded_shape=[128, 512],
                            name=f"ps{m}_{n}",
                        )
                        for n in range(nch)
                    ]
                    for k in range(kt):
                        for n in range(nch):
                            rhs = (
                                src[:, k, ds(tok0 + n * ch, ch)]
                                if src_glob
                                else src[:, k, ds(n * ch, ch)]
                            )
                            nc.tensor.matmul(
                                pss[n],
                                wts[:, k, ts(m, 128)],
                                rhs,
                                start=(k == 0),
                                stop=(k == kt - 1),
                            )
                    for n in range(nch):
                        evict(pss[n], n * ch, ch, m)

                return f

            def ev_h1(ps, o, c, m):
                nc.scalar.activation(
                    ht["h1"][:, m, ds(o, c)], ps, act, bias=b1s[:, m : m + 1]
                )

            def ev_h2(ps, o, c, m):
                nc.scalar.activation(
                    ht["h2"][:, m, ds(o, c)], ps, act, bias=b2s[:, m : m + 1]
                )

            def ev_z(ps, o, c, m):
                nc.scalar.activation(
                    ht["zt"][:, m, ds(o, c)], ps, AF.Identity, bias=b3s[:, m : m + 1]
                )
                nc.scalar.activation(
                    ht["zsq"][:, m, ds(o, c)], ps, AF.Square, bias=b3s[:, m : m + 1]
                )

            units = [mm_unit(xts, True, w1s, KI, m, ev_h1) for m in range(KH)]
            units += [mm_unit(ht["h1"], False, w2s, KH, m, ev_h2) for m in range(KH)]
            units += [mm_unit(ht["h2"], False, w3s, KH, m, ev_z) for m in range(KZ)]

            def nsq_unit(lj):
                def f():
                    nq = nps_p.tile([128, 1], F32, tag="nsq")
                    for k in range(KZ):
                        nc.tensor.matmul(
                            nq,
                            ht["zsq"][:, k, ts(lj, 128)],
                            ones,
                            start=(k == 0),
                            stop=(k == KZ - 1),
                        )
                    sj = small.tile([128, 1], F32, tag="s")
                    nc.scalar.activation(sj, nq, AF.Sqrt, bias=eps2[:, 0:1])
                    nc.vector.reciprocal(ht["rn"][:, lj : lj + 1], sj)

                return f

            units += [nsq_unit(lj) for lj in range(q1 - q0)]
            return units

        # --- head block for one patch pair ----------------------------------
        def head_pair(j):
            hi = next(i for i, (q0, q1) in enumerate(bounds) if q0 <= j < q1)
            lj = j - bounds[hi][0]
            ht = hts[hi]
            va = vt_tiles.pop(2 * j)
            vb = vt_tiles.pop(2 * j + 1)
            ob = None
            for c in range(OUT // 512):
                hp = mmps.tile([128, 512], F32, tag="mm")
                # ABAB order for col-group concurrency (patch A -> psum rows
                # 0:64, patch B -> 64:128 land in distinct col groups of the
                # PE array). Only the very first matmul carries start=True:
                # start clears has_written for the WHOLE bank, so B's first
                # matmul must NOT restart it (that would wipe A's k0 partial);
                # with start=False it overwrites its own rows (bits clear
                # after A's bank-wide start) and accumulates thereafter.
                for k in range(KZ):
                    for half, vv in ((0, va), (64, vb)):
                        nc.tensor.matmul(
                            hp[half : half + 64, :],
                            ht["zt"][:, k, ds(lj * 128 + half, 64)],
                            vv[:, k, ts(c, 512)],
                            start=(k == 0),
                            stop=(k == KZ - 1),
                            skip_group_check=True,
                        )
                if c % 2 == 0:
                    ob = outp.tile([128, 1024], BF16, tag="ob")
                nc.vector.tensor_scalar_mul(
                    ob[:, ts(c % 2, 512)], hp, ht["rn"][:, lj : lj + 1]
                )
                if c % 2 == 1:
                    nc.sync.dma_start(
                        out_d[2 * j : 2 * j + 2, :, ds((c - 1) * 512, 1024)].rearrange(
                            "a b n -> (a b) n"
                        ),
                        ob,
                    )

        # --- emit: software pipeline over sections --------------------------
        # MLP(section 0); then for each section s: its head pairs with
        # MLP(section s+1) units distributed between them as PE filler.
        for u in mlp_units(0):
            u()
        for si, (q0, q1) in enumerate(bounds):
            nxt = mlp_units(si + 1) if si + 1 < len(bounds) else []
            emitted = 0
            for idx, j in enumerate(range(q0, q1)):
                head_pair(j)
                issue_v(2 * (j + 2))
                issue_v(2 * (j + 2) + 1)
                target = round((idx + 1) * len(nxt) / (q1 - q0))
                while emitted < target:
                    nxt[emitted]()
                    emitted += 1

    nc.compile()
    return nc


def shard_inputs(x, W1, b1, W2, b2, W3, b3, v, g, np_patches=NP):
    """Host-side: pad P, fold g/||v|| into v, transpose to device layouts."""
    bf = ml_dtypes.bfloat16
    p_pad = np_patches * N_CORES

    vn = np.sqrt(np.sum(v * v, axis=-1, keepdims=True))
    vs = (v * (g[..., None] / vn)).astype(bf)            # [196, 4096, 256]
    vs_t = np.ascontiguousarray(vs.transpose(0, 2, 1))   # [196, 256, 4096]
    vt_all = np.zeros((p_pad, BT, OUT), dtype=bf)
    vt_all[:P_FULL] = vs_t

    xp = np.zeros((p_pad, B, IN), dtype=np.float32)      # token order (p, b)
    xp[:P_FULL] = x.transpose(1, 0, 2)

    shared = {
        "w1t": np.ascontiguousarray(W1.T).astype(bf),
        "w2t": np.ascontiguousarray(W2.T).astype(bf),
        "w3t": np.ascontiguousarray(W3.T).astype(bf),
        "b1r": np.ascontiguousarray(b1.reshape(-1, 128).T).astype(np.float32),
        "b2r": np.ascontiguousarray(b2.reshape(-1, 128).T).astype(np.float32),
        "b3r": np.ascontiguousarray(b3.reshape(-1, 128).T).astype(np.float32),
    }
    t_tok = np_patches * B
    in_maps = []
    for c in range(N_CORES):
        sl = slice(c * np_patches, (c + 1) * np_patches)
        xt = np.ascontiguousarray(xp[sl].reshape(t_tok, IN).T).astype(bf)
        in_maps.append({"xt": xt, "vt": vt_all[sl], **shared})
    return in_maps


def unshard_output(results, np_patches=NP):
    outs = [np.asarray(r["out"], dtype=np.float32) for r in results]
    full = np.concatenate(outs, axis=0)                  # [208, 64, 4096]
    return np.ascontiguousarray(full[:P_FULL].transpose(1, 0, 2))


_NC_CACHE = {}


def _get_nc():
    if "nc" not in _NC_CACHE:
        _NC_CACHE["nc"] = build_kernel()
    return _NC_CACHE["nc"]


def kernel(x, W1, b1, W2, b2, W3, b3, v, g):
    x = np.asarray(x, dtype=np.float32)
    in_maps = shard_inputs(
        x,
        np.asarray(W1, dtype=np.float32),
        np.asarray(b1, dtype=np.float32),
        np.asarray(W2, dtype=np.float32),
        np.asarray(b2, dtype=np.float32),
        np.asarray(W3, dtype=np.float32),
        np.asarray(b3, dtype=np.float32),
        np.asarray(v, dtype=np.float32),
        np.asarray(g, dtype=np.float32),
    )
    nc = _get_nc()
    res = run_bass_kernel_spmd(nc, in_maps, core_ids=list(range(N_CORES)))
    return unshard_output(res.results)


# revision 14
# speedup vs baseline: 1.5545x; 1.0103x over previous
"""AFiReHead (DINO-style head) distributed Bass kernel for 8 TRN2 NeuronCores.

Reference computation:
    h  = gelu(x @ W1.T + b1)          x: [B=64, P=196, IN=768], W1: [1024, 768]
    h  = gelu(h @ W2.T + b2)          W2: [1024, 1024]
    z  = h @ W3.T + b3                W3: [256, 1024]
    z  = z / max(||z||_2, eps)        over last dim (256)
    W  = g/||v|| * v                  v: [196, 4096, 256] per-patch weight-norm heads
    out[b,p,o] = z[b,p,:] . W[p,o,:]  -> [64, 196, 4096]

Sharding: expert-parallel over the patch axis. P padded 196 -> 208, 26
patches per core; each core runs the full pipeline for its patches with the
small MLP weights replicated. No collectives needed - each core produces an
independent output slice which the host concatenates.

Device layout is feature-major ([feature, token], token = patch*64 + batch)
so activations chain through the three matmuls and the per-patch head with
no on-device transposes; all transposes are host-side numpy views.

Numerics: bf16 matmul operands with f32 PSUM accumulation (verified
rel2err ~5e-3 vs f32 reference). The weight-norm scale g/||v|| is folded
into v host-side (standard inference-time weight-norm folding); the
data-dependent L2 norm of z is computed on device and applied as a
per-partition scale in the head epilogue.

Schedule: the wall-clock floor is the v stream (~54 MB/core). A small MLP
prologue (2 patch pairs) produces the first z tiles, then the remaining MLP
work is emitted as filler units interleaved between per-pair head blocks so
the v DMA stream never stalls on buffer space and the PE never idles long
enough to re-throttle (HAM).
"""

from contextlib import ExitStack

import numpy as np
import ml_dtypes

import concourse.bass as bass
import concourse.tile as tile
from concourse import bacc, mybir
from concourse.bass import ds, ts
from concourse.bass_utils import run_bass_kernel_spmd

BF16 = mybir.dt.bfloat16
F32 = mybir.dt.float32
AF = mybir.ActivationFunctionType

B = 64          # batch
P_FULL = 196    # patches
IN = 768
H = 1024
BT = 256        # bottleneck
OUT = 4096
N_CORES = 8
NP = 25         # patches per core (196 padded to 200)
T = NP * B      # tokens per core, ordered t = p*64 + b


def _nch(tokens):
    n = max(1, (tokens + 511) // 512)
    while tokens % n:
        n += 1
    return n


def build_kernel(
    np_patches=NP,
    num_devices=N_CORES,
    debug=False,
    act=AF.Gelu,
    pairs_a=2,
    v_bufs=4,
    warmup_mms=48,
):
    """Build the per-core Bass graph. All 8 cores run this same graph SPMD."""
    t_tok = np_patches * B
    n_pairs = t_tok // 128          # full 128-token pairs
    single = t_tok % 128 == 64      # trailing unpaired patch
    sections = []
    rem = n_pairs
    while rem:
        s = min(pairs_a if len(sections) < 2 else pairs_a + 1, rem)
        sections.append(s)
        rem -= s
    bounds = []
    q = 0
    for s in sections:
        bounds.append((q, q + s))
        q += s
    KI, KH, KZ = IN // 128, H // 128, BT // 128   # 6, 8, 2 k-tiles

    nc = bacc.Bacc(
        "TRN2", target_bir_lowering=False, debug=debug, num_devices=num_devices
    )
    xt_d = nc.dram_tensor("xt", [IN, t_tok], BF16, kind="ExternalInput").ap()
    w1_d = nc.dram_tensor("w1t", [IN, H], BF16, kind="ExternalInput").ap()
    w2_d = nc.dram_tensor("w2t", [H, H], BF16, kind="ExternalInput").ap()
    w3_d = nc.dram_tensor("w3t", [H, BT], BF16, kind="ExternalInput").ap()
    b1_d = nc.dram_tensor("b1r", [128, KH], F32, kind="ExternalInput").ap()
    b2_d = nc.dram_tensor("b2r", [128, KH], F32, kind="ExternalInput").ap()
    b3_d = nc.dram_tensor("b3r", [128, KZ], F32, kind="ExternalInput").ap()
    vt_d = nc.dram_tensor("vt", [np_patches, BT, OUT], BF16, kind="ExternalInput").ap()
    out_d = nc.dram_tensor("out", [np_patches, B, OUT], BF16, kind="ExternalOutput").ap()

    with tile.TileContext(nc) as tc, ExitStack() as ctx:
        const = ctx.enter_context(tc.tile_pool(name="const", bufs=1))
        acts = ctx.enter_context(tc.tile_pool(name="acts", bufs=1))
        vpool = ctx.enter_context(tc.tile_pool(name="vpool", bufs=v_bufs))
        mmps = ctx.enter_context(tc.tile_pool(name="mmps", bufs=6, space="PSUM"))
        nps_p = ctx.enter_context(tc.tile_pool(name="npsum", bufs=2, space="PSUM"))
        outp = ctx.enter_context(tc.tile_pool(name="outp", bufs=4))
        small = ctx.enter_context(tc.tile_pool(name="small", bufs=2))

        # --- PE warmup: dummy matmuls on constants while inputs stream in ---
        wz = const.tile([128, 512], BF16, name="wz")
        nc.any.memset(wz, 0.0)
        for i in range(warmup_mms):
            wp = mmps.tile([128, 512], F32, tag="mm", name=f"warm{i}")
            nc.tensor.matmul(wp, wz[:, 0:128], wz, start=True, stop=True)

        # --- resident loads -------------------------------------------------
        w1s = const.tile([128, KI, H], BF16)
        nc.sync.dma_start(w1s, w1_d.rearrange("(a p) n -> p a n", p=128))
        xts = acts.tile([128, KI, t_tok], BF16)
        nc.sync.dma_start(xts, xt_d.rearrange("(a p) n -> p a n", p=128))
        b1s = const.tile([128, KH], F32)
        nc.sync.dma_start(b1s, b1_d)
        w2s = const.tile([128, KH, H], BF16)
        nc.sync.dma_start(w2s, w2_d.rearrange("(a p) n -> p a n", p=128))
        w3s = const.tile([128, KH, BT], BF16)
        nc.sync.dma_start(w3s, w3_d.rearrange("(a p) n -> p a n", p=128))
        b2s = const.tile([128, KH], F32)
        nc.sync.dma_start(b2s, b2_d)
        b3s = const.tile([128, KZ], F32)
        nc.sync.dma_start(b3s, b3_d)
        ones = const.tile([128, 1], BF16)
        nc.any.memset(ones, 1.0)
        eps2 = const.tile([128, 1], F32)
        nc.any.memset(eps2, 1e-24)

        # per-section activation tiles (cross-section reads and writes must
        # never share a tile: program order defines semantics under Tile)
        hts = []
        for hi, (q0, q1) in enumerate(bounds):
            tkn = (q1 - q0) * 128 + (64 if single and hi == len(bounds) - 1 else 0)
            hts.append(
                {
                    "h1": acts.tile([128, KH, tkn], BF16, name=f"h1_{hi}"),
                    "h2": acts.tile([128, KH, tkn], BF16, name=f"h2_{hi}"),
                    "zt": acts.tile([128, KZ, tkn], BF16, name=f"zt_{hi}"),
                    "zsq": acts.tile([128, KZ, tkn], BF16, name=f"zsq_{hi}"),
                    "rn": acts.tile(
                        [128, (q1 - q0) + (1 if single and hi == len(bounds) - 1 else 0)],
                        F32,
                        name=f"rn_{hi}",
                    ),
                }
            )

        # --- v stream (per-patch tiles, issued ~2 pairs ahead) --------------
        vt_tiles = {}

        def issue_v(p):
            if p >= np_patches or p in vt_tiles:
                return
            vtl = vpool.tile([128, KZ, OUT], BF16, tag="v", name=f"v{p}")
            nc.sync.dma_start(vtl, vt_d[p].rearrange("(a q) n -> q a n", q=128))
            vt_tiles[p] = vtl

        for p in range(min(v_bufs, np_patches)):
            issue_v(p)

        # --- MLP unit generators (one unit = one m-tile through all k) ------
        def mlp_units(hi):
            q0, q1 = bounds[hi]
            tkn = (q1 - q0) * 128 + (64 if single and hi == len(bounds) - 1 else 0)
            tok0 = q0 * 128
            nch = _nch(tkn)
            ch = tkn // nch
            ht = hts[hi]

            def mm_unit(src, src_glob, wts, kt, m, evict):
                def f():
                    pss = [
                        mmps.tile(
                            [128, ch],
                            F32,
                            tag="mm",
                            padded_shape=[128, 512],
                            name=f"ps{m}_{n}",
                        )
                        for n in range(nch)
                    ]
                    for k in range(kt):
                        for n in range(nch):
                            rhs = (
                                src[:, k, ds(tok0 + n * ch, ch)]
                                if src_glob
                                else src[:, k, ds(n * ch, ch)]
                            )
                            nc.tensor.matmul(
                                pss[n],
                                wts[:, k, ts(m, 128)],
                                rhs,
                                start=(k == 0),
                                stop=(k == kt - 1),
                            )
                    for n in range(nch):
                        evict(pss[n], n * ch, ch, m)

                return f

            def ev_h1(ps, o, c, m):
                nc.scalar.activation(
                    ht["h1"][:, m, ds(o, c)], ps, act, bias=b1s[:, m : m + 1]
                )

            def ev_h2(ps, o, c, m):
                nc.scalar.activation(
                    ht["h2"][:, m, ds(o, c)], ps, act, bias=b2s[:, m : m + 1]
                )

            def ev_z(ps, o, c, m):
                nc.scalar.activation(
                    ht["zt"][:, m, ds(o, c)], ps, AF.Identity, bias=b3s[:, m : m + 1]
                )
                nc.scalar.activation(
                    ht["zsq"][:, m, ds(o, c)], ps, AF.Square, bias=b3s[:, m : m + 1]
                )

            units = [mm_unit(xts, True, w1s, KI, m, ev_h1) for m in range(KH)]
            units += [mm_unit(ht["h1"], False, w2s, KH, m, ev_h2) for m in range(KH)]
            units += [mm_unit(ht["h2"], False, w3s, KH, m, ev_z) for m in range(KZ)]

            def nsq_unit(lj):
                w = min(128, tkn - lj * 128)

                def f():
                    nq = nps_p.tile([128, 1], F32, tag="nsq")
                    for k in range(KZ):
                        nc.tensor.matmul(
                            nq[:w, :],
                            ht["zsq"][:, k, ds(lj * 128, w)],
                            ones,
                            start=(k == 0),
                            stop=(k == KZ - 1),
                        )
                    sj = small.tile([128, 1], F32, tag="s")
                    nc.scalar.activation(sj[:w, :], nq[:w, :], AF.Sqrt, bias=eps2[:w, 0:1])
                    nc.vector.reciprocal(ht["rn"][:w, lj : lj + 1], sj[:w, :])

                return f

            n_nt = (q1 - q0) + (1 if single and hi == len(bounds) - 1 else 0)
            units += [nsq_unit(lj) for lj in range(n_nt)]
            return units

        # --- head block for one patch pair ----------------------------------
        def head_pair(j):
            hi = next(i for i, (q0, q1) in enumerate(bounds) if q0 <= j < q1)
            lj = j - bounds[hi][0]
            ht = hts[hi]
            va = vt_tiles.pop(2 * j)
            vb = vt_tiles.pop(2 * j + 1)
            ob = None
            for c in range(OUT // 512):
                hp = mmps.tile([128, 512], F32, tag="mm")
                # ABAB order for col-group concurrency (patch A -> psum rows
                # 0:64, patch B -> 64:128 land in distinct col groups of the
                # PE array). Only the very first matmul carries start=True:
                # start clears has_written for the WHOLE bank, so B's first
                # matmul must NOT restart it (that would wipe A's k0 partial);
                # with start=False it overwrites its own rows (bits clear
                # after A's bank-wide start) and accumulates thereafter.
                for k in range(KZ):
                    for half, vv in ((0, va), (64, vb)):
                        nc.tensor.matmul(
                            hp[half : half + 64, :],
                            ht["zt"][:, k, ds(lj * 128 + half, 64)],
                            vv[:, k, ts(c, 512)],
                            start=(k == 0),
                            stop=(k == KZ - 1),
                            skip_group_check=True,
                        )
                if c % 2 == 0:
                    ob = outp.tile([128, 1024], BF16, tag="ob")
                nc.vector.tensor_scalar_mul(
                    ob[:, ts(c % 2, 512)], hp, ht["rn"][:, lj : lj + 1]
                )
                if c % 2 == 1:
                    nc.sync.dma_start(
                        out_d[2 * j : 2 * j + 2, :, ds((c - 1) * 512, 1024)].rearrange(
                            "a b n -> (a b) n"
                        ),
                        ob,
                    )

        def head_single():
            p = np_patches - 1
            hi = len(bounds) - 1
            ht = hts[hi]
            lt = (bounds[hi][1] - bounds[hi][0]) * 128   # local token offset
            lcol = bounds[hi][1] - bounds[hi][0]         # rn column
            va = vt_tiles.pop(p)
            ob = None
            for c in range(OUT // 512):
                hp = mmps.tile([128, 512], F32, tag="mm", name="hps")
                for k in range(KZ):
                    nc.tensor.matmul(
                        hp[0:64, :],
                        ht["zt"][:, k, ds(lt, 64)],
                        va[:, k, ts(c, 512)],
                        start=(k == 0),
                        stop=(k == KZ - 1),
                    )
                if c % 2 == 0:
                    ob = outp.tile([128, 1024], BF16, tag="ob", name="obs")
                nc.vector.tensor_scalar_mul(
                    ob[0:64, ts(c % 2, 512)], hp[0:64, :], ht["rn"][0:64, lcol : lcol + 1]
                )
                if c % 2 == 1:
                    nc.sync.dma_start(
                        out_d[p, :, ds((c - 1) * 512, 1024)],
                        ob[0:64, :],
                    )

        # --- emit: software pipeline over sections --------------------------
        # MLP(section 0); then for each section s: its head pairs with
        # MLP(section s+1) units distributed between them as PE filler.
        for u in mlp_units(0):
            u()
        for si, (q0, q1) in enumerate(bounds):
            nxt = mlp_units(si + 1) if si + 1 < len(bounds) else []
            emitted = 0
            for idx, j in enumerate(range(q0, q1)):
                head_pair(j)
                issue_v(2 * (j + 2))
                issue_v(2 * (j + 2) + 1)
                target = round((idx + 1) * len(nxt) / (q1 - q0))
                while emitted < target:
                    nxt[emitted]()
                    emitted += 1
        if single:
            head_single()

    nc.compile()
    return nc


def shard_inputs(x, W1, b1, W2, b2, W3, b3, v, g, np_patches=NP):
    """Host-side: pad P, fold g/||v|| into v, transpose to device layouts."""
    bf = ml_dtypes.bfloat16
    p_pad = np_patches * N_CORES

    vn = np.sqrt(np.sum(v * v, axis=-1, keepdims=True))
    vs = (v * (g[..., None] / vn)).astype(bf)            # [196, 4096, 256]
    vs_t = np.ascontiguousarray(vs.transpose(0, 2, 1))   # [196, 256, 4096]
    vt_all = np.zeros((p_pad, BT, OUT), dtype=bf)
    vt_all[:P_FULL] = vs_t

    xp = np.zeros((p_pad, B, IN), dtype=np.float32)      # token order (p, b)
    xp[:P_FULL] = x.transpose(1, 0, 2)

    shared = {
        "w1t": np.ascontiguousarray(W1.T).astype(bf),
        "w2t": np.ascontiguousarray(W2.T).astype(bf),
        "w3t": np.ascontiguousarray(W3.T).astype(bf),
        "b1r": np.ascontiguousarray(b1.reshape(-1, 128).T).astype(np.float32),
        "b2r": np.ascontiguousarray(b2.reshape(-1, 128).T).astype(np.float32),
        "b3r": np.ascontiguousarray(b3.reshape(-1, 128).T).astype(np.float32),
    }
    t_tok = np_patches * B
    in_maps = []
    for c in range(N_CORES):
        sl = slice(c * np_patches, (c + 1) * np_patches)
        xt = np.ascontiguousarray(xp[sl].reshape(t_tok, IN).T).astype(bf)
        in_maps.append({"xt": xt, "vt": vt_all[sl], **shared})
    return in_maps


def unshard_output(results, np_patches=NP):
    outs = [np.asarray(r["out"], dtype=np.float32) for r in results]
    full = np.concatenate(outs, axis=0)                  # [208, 64, 4096]
    return np.ascontiguousarray(full[:P_FULL].transpose(1, 0, 2))


_NC_CACHE = {}


def _get_nc():
    if "nc" not in _NC_CACHE:
        _NC_CACHE["nc"] = build_kernel()
    return _NC_CACHE["nc"]


def kernel(x, W1, b1, W2, b2, W3, b3, v, g):
    x = np.asarray(x, dtype=np.float32)
    in_maps = shard_inputs(
        x,
        np.asarray(W1, dtype=np.float32),
        np.asarray(b1, dtype=np.float32),
        np.asarray(W2, dtype=np.float32),
        np.asarray(b2, dtype=np.float32),
        np.asarray(W3, dtype=np.float32),
        np.asarray(b3, dtype=np.float32),
        np.asarray(v, dtype=np.float32),
        np.asarray(g, dtype=np.float32),
    )
    nc = _get_nc()
    res = run_bass_kernel_spmd(nc, in_maps, core_ids=list(range(N_CORES)))
    return unshard_output(res.results)
